# revision 14
# baseline (speedup 1.0000x reference)
"""nn_MoE_57492432224434 — MoE (SwiGLU, top-2 of 8 experts) on 8 TRN2 NeuronCores.

Strategy (expert-parallel, matching the sharding hint):
  * Host computes the tiny router (logits = x @ gw.T, top-2, softmax) and
    dispatches tokens: core e receives the tokens routed to expert e
    (transposed, zero-padded to capacity C), plus expert e's weights
    pre-transposed/pre-tiled so every device load is one contiguous DMA.
  * Each core runs a Bass/Tile kernel computing, entirely in float32r
    (full-speed PE mode, ~1e-4 rel err):
        h1T = (x @ w1.T).T ; h2T = (x @ w2.T).T        [PE]
        gT  = silu(h1T) * h2T                          [ACT + DVE]
        yT  = (g @ w3.T).T scaled by combine weight    [PE + DVE]
  * Host scatter-adds each core's yT columns back into the [T, D] output.

All layouts are transposed (tokens on the free axis) so no on-device
transposes are needed anywhere.
"""
import ml_dtypes
import numpy as np
import orjson

import concourse.bass as bass
import concourse.mybir as mybir
import concourse.tile as tile

# ---------------------------------------------------------------------------
# Workaround for this container's walrus build: any instruction carrying more
# than ONE sync-wait command is rejected ("Too many sync wait commands").
# Tile's semaphore assignment routinely attaches several waits to one
# instruction; split the extras onto preceding NOPs on the same engine (same
# basic block, so per-engine program order is preserved).
# ---------------------------------------------------------------------------

def _legalize_bir_json(bir_json: bytes) -> bytes:
    bir = orjson.loads(bir_json)
    for fn in bir.get("functions", []):
        for bb in fn.get("blocks", []):
            out = []
            for inst in bb.get("instructions", []):
                si = inst.get("sync_info")
                waits = si.get("on_wait") if si else None
                if waits and len(waits) > 1:
                    for i, w in enumerate(waits[:-1]):
                        nop = {
                            "engine": inst["engine"], "ins": [], "outs": [],
                            "name": f"{inst['name']}_lw{i}", "opcode": "NoOp",
                            "sync_info": {"on_update": [], "on_wait": [w]},
                        }
                        if "debug" in inst:
                            nop["debug"] = inst["debug"]
                        out.append(nop)
                    si["on_wait"] = [waits[-1]]
                out.append(inst)
            bb["instructions"] = out
    return orjson.dumps(bir)


def _install_legalizer():
    import concourse.bass_utils as bu
    import concourse.bass2jax as b2j
    if getattr(bu.compile_bir_kernel, "_legalized", False):
        return
    orig = bu.compile_bir_kernel

    def wrapped(bir_json, tmpdir, neff_name="file.neff"):
        return orig(_legalize_bir_json(bytes(bir_json)), tmpdir, neff_name=neff_name)

    wrapped._legalized = True
    bu.compile_bir_kernel = wrapped
    b2j.compile_bir_kernel = wrapped


_install_legalizer()

# ---------------------------------------------------------------------------
# Jit-once SPMD runner over axon PJRT (run_bass_kernel_spmd re-jits per call).
# ---------------------------------------------------------------------------

class SpmdRunner:
    def __init__(self, nc, n_cores):
        import jax
        from jax.experimental.shard_map import shard_map
        from jax.sharding import Mesh, PartitionSpec
        import concourse.bass2jax as b2j
        b2j.install_neuronx_cc_hook()
        self.n_cores = n_cores
        partition_name = nc.partition_id_tensor.name if nc.partition_id_tensor else None
        in_names, out_names, out_avals = [], [], []
        for alloc in nc.m.functions[0].allocations:
            if not isinstance(alloc, mybir.MemoryLocationSet):
                continue
            name = alloc.memorylocations[0].name
            if alloc.kind == "ExternalInput":
                if name != partition_name:
                    in_names.append(name)
            elif alloc.kind == "ExternalOutput":
                out_names.append(name)
                out_avals.append(jax.core.ShapedArray(tuple(alloc.tensor_shape),
                                                      mybir.dt.np(alloc.dtype)))
        self.in_names, self.out_names, self.out_avals = in_names, out_names, out_avals
        n_params = len(in_names)

        def _body(*args):
            operands = list(args)
            if partition_name is not None:
                operands.append(b2j.partition_id_tensor())
            outs = b2j._bass_exec_p.bind(
                *operands,
                out_avals=tuple(out_avals),
                in_names=tuple(list(in_names) + list(out_names) +
                               ([partition_name] if partition_name else [])),
                out_names=tuple(out_names),
                lowering_input_output_aliases=(),
                sim_require_finite=False, sim_require_nnan=False, nc=nc,
            )
            return tuple(outs)

        devices = jax.devices()[:n_cores]
        assert len(devices) == n_cores, f"need {n_cores} cores, have {len(devices)}"
        mesh = Mesh(np.asarray(devices), ("core",))
        nz = len(out_names)
        self._fn = jax.jit(
            shard_map(_body, mesh=mesh,
                      in_specs=(PartitionSpec("core"),) * (n_params + nz),
                      out_specs=(PartitionSpec("core"),) * nz,
                      check_rep=False),
            keep_unused=True,
        )
        self._zeros = [
            jax.device_put(np.zeros((n_cores * a.shape[0], *a.shape[1:]), a.dtype))
            for a in out_avals
        ]
        self._jax = jax

    def put_inputs(self, in_maps):
        jax = self._jax
        concat = [
            np.concatenate([np.asarray(in_maps[c][n]) for c in range(self.n_cores)], axis=0)
            for n in self.in_names
        ]
        return [jax.device_put(a) for a in concat]

    def execute(self, dev):
        return self._fn(*dev, *self._zeros)

    def run(self, in_maps):
        outs = [np.asarray(o) for o in self.execute(self.put_inputs(in_maps))]
        return [
            {n: outs[i].reshape(self.n_cores, *self.out_avals[i].shape)[c]
             for i, n in enumerate(self.out_names)}
            for c in range(self.n_cores)
        ]


# ---------------------------------------------------------------------------
# Problem constants (hardcoded per the harness contract) and kernel builder.
# ---------------------------------------------------------------------------

D = 1024          # model dim
F = 2816          # expert hidden dim
E = 8             # experts == cores
TOPK = 2
C_GRAN = 64       # capacity granularity (token axis is the free axis, no 128 need)
C_DEFAULT = 1088  # per-expert token capacity (observed max ~1078)
DT = D // 128
FT = F // 128
FP32R = mybir.dt.float32r
FP32 = mybir.dt.float32
BF16 = mybir.dt.bfloat16


def _tok_tiles(C):
    """Split C into near-equal tiles of <=512 columns (multiples of 64,
    all >=256 when C >= 512) — float32r matmuls drop to 1/4 rate below a
    256-wide moving operand, and wide tiles amortize fixed per-MM cost."""
    n = -(-C // 512)
    tiles, rem = [], C
    for i in range(n, 0, -1):
        t = rem if i == 1 else min(512, -(-rem // i // 64) * 64)
        tiles.append((C - rem, t))
        rem -= t
    return tiles


def build(C, n_copies=1, use_bf16=True):
    WDT = BF16 if use_bf16 else FP32R
    TOK = _tok_tiles(C)
    # phase-B accumulates into one wide PSUM tile (yp); matmul outputs must
    # not cross a PSUM bank boundary (512 fp32), so tile at 512-multiples.
    TOKB = [(t0, min(512, C - t0)) for t0 in range(0, C, 512)]
    YPW = -(-C // 512) * 512  # yp psum tile width, bank-aligned
    nc = bass.Bass(target_bir_lowering=False)
    xt = nc.dram_tensor("xt", [D, C], WDT, kind="ExternalInput")
    # weights arrive pre-packed as per-iteration SBUF images:
    # w1p/w2p[f] = [128 partitions, DT*128] with col k*128+m = w[f*128+m, k*128+p]
    w1p = nc.dram_tensor("w1p", [FT, 128, DT * 128], WDT, kind="ExternalInput")
    w2p = nc.dram_tensor("w2p", [FT, 128, DT * 128], WDT, kind="ExternalInput")
    w3p = nc.dram_tensor("w3p", [DT, 128, FT * 128], WDT, kind="ExternalInput")
    cw = nc.dram_tensor("cw", [128, C], FP32, kind="ExternalInput")
    yt = nc.dram_tensor("yt", [D, C], FP32, kind="ExternalOutput")

    with tile.TileContext(nc) as tc:
        with (
            tc.tile_pool(name="resident", bufs=1) as rpool,
            tc.tile_pool(name="stream", bufs=2) as spool,
            tc.tile_pool(name="work", bufs=2) as wpool,
            tc.tile_pool(name="psum", bufs=1, space="PSUM") as ppool,
        ):
          for _copy in range(n_copies):
            xsb = rpool.tile([128, DT * C], WDT, tag="xsb")  # x.T: d-chunk k at cols [k*C, (k+1)*C)
            gsb = rpool.tile([128, FT * C], WDT, tag="gsb")  # g.T: f-chunk f at cols [f*C, (f+1)*C)
            cwsb = rpool.tile([128, C], FP32, tag="cwsb")
            nc.sync.dma_start(out=cwsb[:, :], in_=cw[:, :])
            for k in range(DT):
                nc.sync.dma_start(out=xsb[:, bass.ds(k * C, C)], in_=xt[k*128:(k+1)*128, :])

            # phase A: gT = silu((x @ w1.T).T) * (x @ w2.T).T
            for f in range(FT):
                w1sb = spool.tile([128, DT * 128], WDT, tag="w1sb")
                w2sb = spool.tile([128, DT * 128], WDT, tag="w2sb")
                nc.sync.dma_start(out=w1sb[:, :], in_=w1p[f])
                nc.sync.dma_start(out=w2sb[:, :], in_=w2p[f])
                for (t0, tn) in TOK:
                    h1 = ppool.tile([128, 512], FP32, tag="h1", bufs=1)
                    h2 = ppool.tile([128, 512], FP32, tag="h2", bufs=1)
                    for k in range(DT):
                        nc.tensor.matmul(h1[:, :tn], w1sb[:, bass.ts(k, 128)],
                                         xsb[:, bass.ds(k * C + t0, tn)],
                                         start=(k == 0), stop=(k == DT - 1))
                    for k in range(DT):
                        nc.tensor.matmul(h2[:, :tn], w2sb[:, bass.ts(k, 128)],
                                         xsb[:, bass.ds(k * C + t0, tn)],
                                         start=(k == 0), stop=(k == DT - 1))
                    smu = wpool.tile([128, 512], WDT, tag="smu")
                    nc.scalar.activation(smu[:, :tn], h1[:, :tn],
                                         mybir.ActivationFunctionType.Silu)
                    nc.vector.tensor_mul(gsb[:, bass.ds(f * C + t0, tn)],
                                         smu[:, :tn], h2[:, :tn])

            # phase B: yT[d,:] = sum_f w3t-block.T @ gT, scaled by combine weight
            for d in range(DT):
                w3sb = spool.tile([128, FT * 128], WDT, tag="w3sb")
                nc.sync.dma_start(out=w3sb[:, :], in_=w3p[d])
                yp = ppool.tile([128, YPW], FP32, tag="yp", bufs=2)
                for f in range(FT):
                    for (t0, tn) in TOKB:
                        nc.tensor.matmul(yp[:, bass.ds(t0, tn)], w3sb[:, bass.ts(f, 128)],
                                         gsb[:, bass.ds(f * C + t0, tn)],
                                         start=(f == 0), stop=(f == FT - 1))
                osb = wpool.tile([128, C], FP32, tag="osb", bufs=2)
                nc.vector.tensor_mul(osb[:, :], yp[:, :C], cwsb[:, :])
                nc.sync.dma_start(out=yt[d*128:(d+1)*128, :], in_=osb[:, :])
    return nc


# ---------------------------------------------------------------------------
# Host routing / dispatch / combine
# ---------------------------------------------------------------------------

def _route(x, gw):
    logits = x @ gw.T                                    # [T, E]
    order = np.argsort(-logits, axis=1, kind="stable")   # ties -> lower idx, as top_k
    idx = order[:, :TOPK]
    vals = np.take_along_axis(logits, idx, axis=1)
    ex = np.exp(vals - vals[:, :1])
    sv = ex / ex.sum(axis=1, keepdims=True)
    per_expert = []
    for e in range(E):
        mask = idx == e
        tok = np.nonzero(mask.any(axis=1))[0]
        per_expert.append((tok, sv[mask]))
    return per_expert


_runners = {}


def _get_runner(C):
    if C not in _runners:
        _runners[C] = SpmdRunner(build(C), E)
    return _runners[C]


def make_in_maps(x, gw, w1, w2, w3, use_bf16=True):
    """Route tokens and pack per-core device inputs. Returns (in_maps, per_expert, C)."""
    wt = ml_dtypes.bfloat16 if use_bf16 else np.float32
    per_expert = _route(x, gw)
    max_n = max(len(tok) for tok, _ in per_expert)
    C = max(256, -(-max_n // C_GRAN) * C_GRAN)
    in_maps = []
    for e in range(E):
        tok, w = per_expert[e]
        n = len(tok)
        xt = np.zeros((D, C), wt)
        xt[:, :n] = x[tok].T.astype(wt)
        cwrow = np.zeros((1, C), np.float32)
        cwrow[0, :n] = w
        # pack weights as the exact SBUF images the kernel loads, so each
        # device DMA is one contiguous [128, n*128] block per iteration:
        #   w1p[f, p, k*128+m] = w1[e][f*128+m, k*128+p]   (w1[e]: [F, D])
        #   w3p[d, p, f*128+m] = w3[e][d*128+m, f*128+p]   (w3[e]: [D, F])
        in_maps.append({
            "xt": xt,
            "w1p": np.ascontiguousarray(
                w1[e].reshape(FT, 128, DT, 128).transpose(0, 3, 2, 1)
                .reshape(FT, 128, DT * 128).astype(wt)),
            "w2p": np.ascontiguousarray(
                w2[e].reshape(FT, 128, DT, 128).transpose(0, 3, 2, 1)
                .reshape(FT, 128, DT * 128).astype(wt)),
            "w3p": np.ascontiguousarray(
                w3[e].reshape(DT, 128, FT, 128).transpose(0, 3, 2, 1)
                .reshape(DT, 128, FT * 128).astype(wt)),
            "cw": np.ascontiguousarray(np.broadcast_to(cwrow, (128, C))),
        })
    return in_maps, per_expert, C


def kernel(xmat, gw, w1, w2, w3):
    B, L, d = xmat.shape
    x = np.ascontiguousarray(np.asarray(xmat, dtype=np.float32).reshape(-1, d))
    gw = np.asarray(gw, dtype=np.float32)
    w1 = np.asarray(w1, dtype=np.float32)
    w2 = np.asarray(w2, dtype=np.float32)
    w3 = np.asarray(w3, dtype=np.float32)

    in_maps, per_expert, C = make_in_maps(x, gw, w1, w2, w3)
    results = _get_runner(C).run(in_maps)

    y = np.zeros((x.shape[0], D), np.float32)
    for e in range(E):
        tok, _ = per_expert[e]
        y[tok] += results[e]["yt"][:, :len(tok)].T
    return y.reshape(B, L, d)

